# revision 7
# baseline (speedup 1.0000x reference)
"""DKVMN (DeepIRT) forward pass on 8 Trainium2 NeuronCores.

Strategy
--------
Pure data parallel over the batch (2048 -> 256 per core, two 128-row
halves). Token-dependent quantities are folded into gather tables on the
host (weight-only preprocessing):

  Wsoft[q]  = softmax(q_embed @ key_memory^T)   (attention weights w)
  Hq[q]     = q_embed @ pred_w1[V:] + b1        (query part of the MLP)
  Esig[qa]  = sigmoid(qa_embed @ erase_w + be)  (erase gate e)
  Atanh[qa] = tanh(qa_embed @ add_w + ba)       (add vector a)

Value memory lives SBUF-resident as Mv[p=128, V, M] per batch half
(m innermost, fp16). Per step and half:

  DVE   : T1 = Mv * w_bc          (w broadcast along V, packed innermost)
          read = sum_m T1         (in-place add-tree over m, fp16 2x mode)
          Mv += X                 (X = w (x) a, prefetched on GPSIMD)
          T1 *= ER; Mv -= T1      (erase; ER = e replicated along m)
  ACT   : ER for step t+1 (broadcast copy), tanh/sigmoid epilogue
  GPSIMD: X = w (x) a for step t+1 in two 25-slot halves (3 rotating
          buffers per batch half -> zero-stall prefetch), dma_gather rows
  PE    : read transposes + prediction matmuls

Env toggles (build-time): KERNEL_FUSED=1 fuse both halves into 3-free-dim
ops; KERNEL_REDUCE1=1 single tensor_reduce instead of the tree;
KERNEL_PREDPE=1 fold hq via identity matmul + tensor_tensor_reduce;
KERNEL_XH_DVE=1 outer product on DVE instead of GPSIMD.
"""

import os
import sys

for _p in ("/root/.axon_site/_ro/trn_rl_repo", "/opt/trn_rl_repo"):
    if os.path.isdir(_p) and _p not in sys.path:
        sys.path.append(_p)

import numpy as np

import concourse.bacc as bacc
import concourse.bass as bass
import concourse.tile as tile
from concourse import mybir
from concourse.bass_utils import run_bass_kernel_spmd
from concourse.masks import make_identity

# Problem shapes (hardcoded per harness contract)
B, S, M, V, KD, FC = 2048, 200, 50, 200, 50, 50
NQ, NQA = 5001, 10001
NCORES = 8
BL = B // NCORES      # 256 batch rows per core
P = 128               # SBUF partitions
NT = BL // P          # 2 batch halves per core
MH = M // 2           # outer-product half width
KSTEPS = 2            # time steps per gather block
NBLK = S // KSTEPS
EAW = 512             # ea-table row width (fp16 elems); e@0:200, a@256:456
WHW = 128             # wh-table row width; w@0:50, hq@64:114
IDX_PER_BLK = BL * KSTEPS        # 512 gathered rows per block per table
IDXCOLS = BL * S // 16           # wrapped idx array columns

_prog_cache = {}


def _flags():
    return tuple(
        bool(int(os.environ.get(k, "0")))
        for k in ("KERNEL_FUSED", "KERNEL_REDUCE1", "KERNEL_PREDPE", "KERNEL_XH_DVE")
    )


def _build_program(steps=S):
    fused, reduce1, predpe, xh_dve = _flags()
    dt = mybir.dt
    nc = bacc.Bacc("TRN2", debug=False)

    ea_t = nc.dram_tensor("ea_table", [NQA, EAW], dt.float16, kind="ExternalInput")
    wh_t = nc.dram_tensor("wh_table", [NQ, WHW], dt.float16, kind="ExternalInput")
    w1r_d = nc.dram_tensor("w1r", [2, 100, FC], dt.float16, kind="ExternalInput")
    w2_d = nc.dram_tensor("w2rep", [P, FC], dt.float16, kind="ExternalInput")
    b2_d = nc.dram_tensor("b2rep", [P, 1], dt.float32, kind="ExternalInput")
    mv_d = nc.dram_tensor("mv_init", [1, V * M], dt.float16, kind="ExternalInput")
    qi_d = nc.dram_tensor("qidx", [P, IDXCOLS], dt.int16, kind="ExternalInput")
    qa_d = nc.dram_tensor("qaidx", [P, IDXCOLS], dt.int16, kind="ExternalInput")
    preds_d = nc.dram_tensor("preds_out", [BL, S], dt.float32, kind="ExternalOutput")

    nblk = steps // KSTEPS

    from contextlib import ExitStack

    mult = mybir.AluOpType.mult
    addop = mybir.AluOpType.add

    with tile.TileContext(nc) as tc, ExitStack() as ctx:
        consts = ctx.enter_context(tc.tile_pool(name="consts", bufs=1))
        state = ctx.enter_context(tc.tile_pool(name="state", bufs=1))
        gath = ctx.enter_context(tc.tile_pool(name="gath", bufs=2))
        small = ctx.enter_context(tc.tile_pool(name="small", bufs=3))
        psum = ctx.enter_context(tc.tile_pool(name="psum", bufs=2, space="PSUM"))

        # ---- constants ----
        w1r_sb = consts.tile([100, 2, FC], dt.float16)
        for c in range(2):
            nc.sync.dma_start(out=w1r_sb[:, c, :], in_=w1r_d[c])
        w2_sb = consts.tile([P, FC], dt.float16)
        nc.sync.dma_start(out=w2_sb[:], in_=w2_d[:])
        b2_sb = consts.tile([P, 1], dt.float32)
        nc.sync.dma_start(out=b2_sb[:], in_=b2_d[:])
        ident = consts.tile([P, P], dt.float16)
        make_identity(nc, ident)

        # ---- persistent state (per-partition SBUF: Mv/T1/ER 2x20KB each,
        #      XH 2x3x10KB -> ~180KB total) ----
        Mvs, T1s, ERs = [], [], []
        XHs = []
        for bt in range(NT):
            Mv = state.tile([P, V, M], dt.float16, tag=f"mv{bt}", name=f"mv{bt}")
            nc.sync.dma_start(
                out=Mv[:].rearrange("p v m -> p (v m)"),
                in_=mv_d[:].to_broadcast((P, V * M)),
            )
            Mvs.append(Mv)
            T1s.append(state.tile([P, V, M], dt.float16, tag=f"t1{bt}", name=f"t1{bt}"))
            ERs.append(state.tile([P, V, M], dt.float16, tag=f"er{bt}", name=f"er{bt}"))
            XHs.append([
                state.tile([P, V, MH], dt.float16, tag=f"xh{bt}_{i}", name=f"xh{bt}_{i}")
                for i in range(3)
            ])
        read = state.tile([P, NT, V], dt.float16, tag="read")
        preds_buf = state.tile([P, NT, S], dt.float32, tag="preds")

        xh_eng = nc.vector if xh_dve else nc.gpsimd

        def load_block(g):
            qi = gath.tile([P, IDX_PER_BLK // 16], dt.int16, tag="qi")
            qa = gath.tile([P, IDX_PER_BLK // 16], dt.int16, tag="qa")
            c0 = g * (IDX_PER_BLK // 16)
            nc.sync.dma_start(out=qi[:], in_=qi_d[:, c0:c0 + IDX_PER_BLK // 16])
            nc.sync.dma_start(out=qa[:], in_=qa_d[:, c0:c0 + IDX_PER_BLK // 16])
            ea_blk = gath.tile([P, KSTEPS, NT, EAW], dt.float16, tag="ea")
            wh_blk = gath.tile([P, KSTEPS, NT, WHW], dt.float16, tag="wh")
            nc.gpsimd.dma_gather(
                ea_blk[:].rearrange("p k a w -> p (k a) w"),
                ea_t[:], qa[:], IDX_PER_BLK, IDX_PER_BLK, EAW)
            nc.gpsimd.dma_gather(
                wh_blk[:].rearrange("p k a w -> p (k a) w"),
                wh_t[:], qi[:], IDX_PER_BLK, IDX_PER_BLK, WHW)
            return ea_blk, wh_blk

        def emit_er(blk, k):
            # ER = e replicated along innermost m (ACT broadcast copy)
            ea_blk, _ = blk
            for bt in range(NT):
                e_sl = ea_blk[:, k, bt, 0:V]
                nc.scalar.copy(
                    ERs[bt][:], e_sl[:, :, None].to_broadcast((P, V, M)))

        def emit_xh(blk, t):
            # X = w (x) a outer product for step t, two m-halves
            ea_blk, wh_blk = blk
            k = t % KSTEPS
            for bt in range(NT):
                a_sl = ea_blk[:, k, bt, 256:256 + V]
                for h in range(2):
                    xb = XHs[bt][(2 * t + h) % 3]
                    w_sl = wh_blk[:, k, bt, h * MH:(h + 1) * MH]
                    xh_eng.tensor_mul(
                        xb[:],
                        w_sl[:, None, :].to_broadcast((P, V, MH)),
                        a_sl[:, :, None].to_broadcast((P, V, MH)),
                    )

        def emit_tree(bt):
            # in-place add-tree over innermost m on T1 -> read[:, bt, :]
            T1 = T1s[bt]
            for lo, hi in ((25, 50), (12, 24), (6, 12), (3, 6)):
                nc.vector.tensor_add(
                    T1[:, :, 0:hi - lo], T1[:, :, 0:hi - lo], T1[:, :, lo:hi])
            # now slots {0,1,2} hold partials of m in 0..24 except slot 24
            nc.vector.tensor_add(T1[:, :, 0:1], T1[:, :, 0:1], T1[:, :, 1:2])
            nc.vector.tensor_add(T1[:, :, 0:1], T1[:, :, 0:1], T1[:, :, 2:3])
            nc.vector.tensor_add(
                read[:, bt, :][:, :, None], T1[:, :, 0:1], T1[:, :, 24:25])

        # ---- prologue: block 0+1 gathers, step-0 ER/X ----
        cur = load_block(0)
        nxt = load_block(1) if nblk > 1 else None
        emit_er(cur, 0)
        emit_xh(cur, 0)

        # ---- scan ----
        for g in range(nblk):
            for k in range(KSTEPS):
                t = g * KSTEPS + k
                ea_blk, wh_blk = cur

                for bt in range(NT):
                    Mv, T1, ER = Mvs[bt], T1s[bt], ERs[bt]
                    w_sl = wh_blk[:, k, bt, 0:M]
                    # DVE: T1 = Mv * w_bc (pre-update memory, weighted)
                    nc.vector.tensor_mul(
                        T1[:], Mv[:], w_sl[:, None, :].to_broadcast((P, V, M)))
                    # DVE: Mv += w (x) a (prefetched halves; frees XH bufs)
                    for h in range(2):
                        xb = XHs[bt][(2 * t + h) % 3]
                        msl = Mv[:, :, h * MH:(h + 1) * MH]
                        nc.vector.tensor_add(msl, msl, xb[:])
                    # DVE: erase: T1c = T1 * ER; Mv -= T1c.  T1c goes into
                    # ER in-place so T1 survives for the read reduction.
                    nc.vector.tensor_mul(ER[:], T1[:], ER[:])
                    nc.vector.tensor_sub(Mv[:], Mv[:], ER[:])
                    # DVE: read = sum_m T1
                    if reduce1:
                        with nc.allow_low_precision(reason="fp16 m-reduce"):
                            nc.vector.tensor_reduce(
                                read[:, bt, :], T1[:], mybir.AxisListType.X, addop)
                    else:
                        emit_tree(bt)

                # next step's ER (ACT) and X halves (GPSIMD); emitted after
                # this step's erase (program order defines the ER dep)
                if t + 1 < steps:
                    blk_next = cur if k + 1 < KSTEPS else nxt
                    emit_er(blk_next, (t + 1) % KSTEPS)
                    emit_xh(blk_next, t + 1)

                # ---- prediction MLP (PE/ACT, off critical path) ----
                for bt in range(NT):
                    h_ps = psum.tile([P, FC], dt.float32, tag=f"hps{bt}")
                    readT = small.tile([100, 2, P], dt.float16, tag=f"rT{bt}")
                    for cc in range(2):
                        pT = psum.tile([100, P], dt.float16, tag=f"pT{bt}")
                        nc.tensor.transpose(
                            pT[:], read[:, bt, cc * 100:(cc + 1) * 100], ident[:])
                        nc.scalar.copy(readT[:, cc, :], pT[:])
                    nc.tensor.matmul(h_ps[:], lhsT=readT[:, 0, :], rhs=w1r_sb[:, 0, :],
                                     start=True, stop=predpe and False)
                    hq_sl = wh_blk[:, k, bt, 64:64 + FC]
                    if predpe:
                        nc.tensor.matmul(h_ps[:], lhsT=readT[:, 1, :],
                                         rhs=w1r_sb[:, 1, :], start=False, stop=False)
                        # += hq via identity matmul into the same PSUM bank
                        nc.tensor.matmul(h_ps[:], lhsT=ident[:], rhs=hq_sl,
                                         start=False, stop=True)
                        hact = small.tile([P, FC], dt.float16, tag=f"hact{bt}")
                        nc.scalar.activation(hact[:], h_ps[:],
                                             mybir.ActivationFunctionType.Tanh)
                        hw2 = small.tile([P, FC], dt.float16, tag=f"hw2{bt}")
                        pacc = small.tile([P, 1], dt.float32, tag=f"pacc{bt}")
                        nc.vector.tensor_tensor_reduce(
                            out=hw2[:], in0=hact[:], in1=w2_sb[:], scale=1.0,
                            scalar=0.0, op0=mult, op1=addop, accum_out=pacc[:])
                    else:
                        nc.tensor.matmul(h_ps[:], lhsT=readT[:, 1, :],
                                         rhs=w1r_sb[:, 1, :], start=False, stop=True)
                        hpre = small.tile([P, FC], dt.float16, tag=f"hpre{bt}")
                        nc.vector.tensor_add(hpre[:], h_ps[:], hq_sl)
                        hact = small.tile([P, FC], dt.float16, tag=f"hact{bt}")
                        nc.scalar.activation(hact[:], hpre[:],
                                             mybir.ActivationFunctionType.Tanh)
                        hw2 = small.tile([P, FC], dt.float16, tag=f"hw2{bt}")
                        pacc = small.tile([P, 1], dt.float32, tag=f"pacc{bt}")
                        nc.vector.tensor_mul(hw2[:], hact[:], w2_sb[:])
                        nc.vector.tensor_reduce(
                            pacc[:], hw2[:], mybir.AxisListType.X, addop)
                    nc.scalar.activation(
                        preds_buf[:, bt, t:t + 1], pacc[:],
                        mybir.ActivationFunctionType.Sigmoid, bias=b2_sb[:])

            # rotate gather blocks; prefetch block g+2
            if g + 1 < nblk:
                new_nxt = load_block(g + 2) if g + 2 < nblk else None
                cur = nxt
                nxt = new_nxt

        # ---- write out ----
        pv = preds_d[:].rearrange("(n p) s -> n p s", p=P)
        for bt in range(NT):
            nc.sync.dma_start(out=pv[bt][:, 0:steps], in_=preds_buf[:, bt, 0:steps])

    nc.finalize()
    return nc


def _wrap_idx(seq):
    """seq [N] -> [128, N//16] int16 wrapped (idx i at [i%16, i//16], 8x replicated)."""
    n = seq.shape[0]
    arr16 = seq.reshape(n // 16, 16).T.astype(np.int16)
    return np.tile(arr16, (8, 1))


def _host_tables(inputs):
    f32 = np.float32
    qe = inputs["q_embed_w"].astype(f32)
    qae = inputs["qa_embed_w"].astype(f32)
    km = inputs["key_memory"].astype(f32)

    logits = qe @ km.T
    ex = np.exp(logits - logits.max(-1, keepdims=True))
    wsoft = ex / ex.sum(-1, keepdims=True)
    hq = qe @ inputs["pred_w1"][V:, :].astype(f32) + inputs["pred_b1"].astype(f32)
    esig = 1.0 / (1.0 + np.exp(-(qae @ inputs["erase_w"].astype(f32) + inputs["erase_b"].astype(f32))))
    atanh = np.tanh(qae @ inputs["add_w"].astype(f32) + inputs["add_b"].astype(f32))

    ea = np.zeros((NQA, EAW), np.float16)
    ea[:, 0:V] = esig.astype(np.float16)
    ea[:, 256:256 + V] = atanh.astype(np.float16)
    wh = np.zeros((NQ, WHW), np.float16)
    wh[:, 0:M] = wsoft.astype(np.float16)
    wh[:, 64:64 + FC] = hq.astype(np.float16)

    w1r = inputs["pred_w1"][:V, :].astype(np.float16).reshape(2, 100, FC)
    w2rep = np.tile(inputs["pred_w2"][:, 0].astype(np.float16)[None, :], (P, 1))
    b2rep = np.full((P, 1), inputs["pred_b2"][0], np.float32)
    # value memory in [V, M] (m innermost) layout
    mv_init = inputs["init_value_memory"].astype(np.float16).T.reshape(1, -1)
    return dict(ea_table=ea, wh_table=wh, w1r=w1r, w2rep=w2rep, b2rep=b2rep,
                mv_init=mv_init)


def kernel(**inputs):
    inputs = {k: np.asarray(v) for k, v in inputs.items()}
    steps = int(os.environ.get("KERNEL_STEPS", S))

    key = (steps, _flags())
    if key not in _prog_cache:
        _prog_cache[key] = _build_program(steps)
    nc = _prog_cache[key]

    shared = _host_tables(inputs)
    q = inputs["q_data"].astype(np.int64)
    qa = inputs["qa_data"].astype(np.int64)

    in_maps = []
    for core in range(NCORES):
        qs = q[core * BL:(core + 1) * BL]       # [256, S]
        qas = qa[core * BL:(core + 1) * BL]
        # gather order: block g, step k, half bt, partition p
        #   -> element (g*K + k) of column (bt*128+p)
        def order(x):
            # x [BL, S] -> [S, NT, P] -> [NBLK, KSTEPS, NT, P] flat
            xt = x.T.reshape(S, NT, P)
            return xt.reshape(NBLK, KSTEPS, NT, P).reshape(-1)
        m = dict(shared)
        m["qidx"] = _wrap_idx(order(qs))
        m["qaidx"] = _wrap_idx(order(qas))
        in_maps.append(m)

    trace = bool(int(os.environ.get("KERNEL_TRACE", "0")))
    res = run_bass_kernel_spmd(nc, in_maps, core_ids=list(range(NCORES)), trace=trace)
    global LAST_RESULTS
    LAST_RESULTS = res
    preds = np.concatenate(
        [res.results[i]["preds_out"] for i in range(NCORES)], axis=0
    ).astype(np.float32)
    z = np.zeros_like(preds)
    return (preds, z, z, z)
